# revision 16
# baseline (speedup 1.0000x reference)
"""Trainium2 Bass kernel for nn_Decoder (RBF decoder).

Math (shapes: t (4,512,1), z (4,512,128), x (4,512,1), sigma (128,),
W (2,128), b (2,)):
    diff[b,n,m] = x[b,m] - t[b,n]                  (XD=1, sum(-1) trivial)
    K[b,n,m,c]  = exp(-0.5 * (diff/exp(sigma[c]))^2)
    y[b,m,c]    = sum_n z[b,n,c] * K[b,n,m,c]
    out[b,m,:]  = y[b,m,:] @ W.T + b

When all sigma[c] are equal (they are zeros for this problem), K is
channel-independent, so W can be folded into z up front:
    zw[b] = z[b] @ W.T            (host, (N,2) per batch — tiny)
    out[b].T = sum_n zw[b,n,:]^T K[b][n,:],  K[b] = exp(s * (x_m - t_n)^2),
    s = -0.5*exp(-2*sigma).

Device mapping (8 cores, SPMD): core k handles batch b=k//2, n-half
h=k%2 (n-slice of 256 = 2 tiles of 128 partitions). Per core:
  - d2[n,m] = (x_m - t_n)^2 is produced directly in PSUM by a rank-3
    bf16 matmul: lhsT = [t^2; -2t; 1] (3,128 per n-tile), rhs =
    [1; x; x^2] (3,512), so no x-broadcast DMA and no Square pass.
    Host pre-rounds everything to bf16; products are exact in the fp32
    PSUM accumulator, so the only error is input rounding (~3e-3 rel
    on the final output, vs the 2e-2 gate).
  - ScalarE: K = exp(s * d2) read straight from PSUM, written to SBUF
    as bf16 (s baked as the ACT scale immediate). The ACT table load
    (~1.3us) is hoisted by the engine queue to run during the input
    DMA window.
  - PE: psum(2,512) += matmul(lhsT=zw bf16 (128,2), rhs=K bf16
    (128,512)) accumulated over the 2 n-tiles. bf16 single-pass
    matmuls (fp32 would be LOW_HIGH dual-issue, ~2x the cost). No
    HAM warm-up: with only 4 matmuls the cold-clock penalty is far
    smaller than the ~4.5us a warm-up string costs.
  - DVE evicts the psum -> SBUF, one DMA out (2,512) = out[b].T
    partial.
Host sums the two n-half partials per batch, transposes, adds bias b.

Input DMAs ride the two independent HWDGE rings (nc.sync / nc.scalar)
so they overlap; nothing touches the GpSimd SWDGE path (its drain tail
is ~5us).

Sync-wait discipline: this container's walrus allows a single on_wait
per instruction ("Too many sync wait commands"), so _split_multi_waits
rewrites the scheduled BIR, hoisting extra waits onto same-engine NOPs
placed immediately before the instruction (same-engine program order
preserves semantics).

General (non-uniform) sigma falls back to grouping channels by unique
sigma value (zw_g from just that group's channels, s_g baked into a
per-group NEFF) and summing the group outputs, which is exact since the
output is linear in z. The graded instance has sigma == 0: one group.
"""

import numpy as np

B, N, M, C, Y = 4, 512, 512, 128, 2
NHALF = N // 2  # n-slice per core
NT = NHALF // 128  # n-tiles of 128 per core

_CACHE = {}


def _split_multi_waits(nc):
    import concourse.mybir as mybir

    for fn in nc.m.functions:
        for blk in fn.blocks:
            il = blk.instructions
            new = []
            for inst in il:
                si = inst.sync_info
                if si is not None and si.on_wait is not None and len(si.on_wait) > 1:
                    waits = list(si.on_wait)
                    for j, w in enumerate(waits[:-1]):
                        new.append(
                            mybir.InstNoOp(
                                name=f"{inst.name}-w{j}",
                                engine=inst.engine,
                                sync_info=mybir.SyncInfo(on_wait=[w], on_update=[]),
                                bass_nofuse=True,
                            )
                        )
                    si.on_wait = [waits[-1]]
                    inst.sync_info = si
                new.append(inst)
            il[:] = new


def _restructure(nc, dma_insts):
    """Post-build BIR surgery to pull fixed latency off the critical path.

    1. Hoist the input DMAs to the very FRONT of their engine's stream
       in the 'main' entry block. The walrus NEFF prologue (start
       barrier + register loads, ~5.6us) runs per engine ahead of
       'main'; dispatching the DMAs first overlaps their ~2.2us
       fixed descriptor/doorbell/HBM latency with the rest of the
       entry sequence instead of paying it serially in the body.
    2. Drop the TileContext entry barrier (per-engine InstDrain +
       EventSemaphore handshake on S151/S152). It only ordered the
       Pool const-tile memsets against the body; the first consumer
       (the exp bias read) runs >2us after the memsets regardless,
       and the input DMAs must not sit behind a Drain (an InstDrain
       waits for the engine's outstanding DMA descriptors to retire).
    3. Drop the end-block handshake that FOLLOWS the semaphore-reset
       InstISA (round B). The walrus end-of-iteration all-engine
       barrier already orders the resets before the next execution.

    Iteration safety: the walrus inter-iteration barrier keeps the
    hoisted DMA writes of run N+1 after all reads of run N, and the
    kept pre-reset handshake (round A) still quiesces every engine —
    including the output DMA receipt — before the InstISA resets.
    """
    import concourse.mybir as mybir

    fn = nc.m.functions[0]
    main, end = fn.blocks[0], fn.blocks[-1]
    dma = [i.ins if hasattr(i, "ins") else i for i in dma_insts]
    names = {i.name for i in dma}
    for blk in fn.blocks:
        blk.instructions[:] = [i for i in blk.instructions if i.name not in names]
    main.instructions[:] = [
        i
        for i in main.instructions
        if not isinstance(i, (mybir.InstDrain, mybir.InstEventSemaphore))
    ]
    il = main.instructions
    for inst in reversed(dma):
        si = inst.sync_info
        assert si is None or not si.on_wait, f"hoisted DMA has waits: {inst.name}"
        idx = next(j for j, m in enumerate(il) if m.engine == inst.engine)
        il.insert(idx, inst)
    # End block: keep only the SP stream up to and including its first
    # InstDrain (the multi-wait drain gating NEFF completion on the
    # output-DMA receipt). The pre-reset handshake, the InstISA
    # semaphore resets, and the post-reset handshake all go: the
    # runtime resets every semaphore between executions anyway, and
    # the walrus end-of-iteration all-engine barrier orders runs.
    il = end.instructions
    kept = []
    sp_done = False
    for m in il:
        if str(m.engine).endswith("SP") and not sp_done:
            kept.append(m)
            if isinstance(m, mybir.InstDrain):
                sp_done = True
    il[:] = kept


def build_bass(s: float):
    """Build the per-core Bass module; `s` (= -0.5*exp(-2*sigma)) is baked
    into the exp activation as a float immediate."""
    import concourse.bass as bass
    import concourse.mybir as mybir
    import concourse.tile as tile

    f32 = mybir.dt.float32
    bf16 = mybir.dt.bfloat16
    nc = bass.Bass(enable_partition_id=False)
    # s3 = [lhsT3 tile0 | lhsT3 tile1 | rhs3]: rows [t^2;-2t;1 | 1;x;x^2]
    s3 = nc.dram_tensor("s3", (3, NT * 128 + M), bf16, kind="ExternalInput")
    # zw cols per n-tile: (128, NT*Y)
    zw = nc.dram_tensor("zw", (128, NT * Y), bf16, kind="ExternalInput")
    o = nc.dram_tensor("o", (Y, M), f32, kind="ExternalOutput")

    with tile.TileContext(nc) as tc:
        with (
            tc.tile_pool(name="const", bufs=1) as cpool,
            tc.tile_pool(name="work", bufs=2) as work,
            tc.tile_pool(name="dpsum", bufs=2, space="PSUM") as dpsum,
            tc.tile_pool(name="opsum", bufs=1, space="PSUM") as opsum,
            tc.tile_pool(name="wpsum", bufs=1, space="PSUM") as wpsum,
        ):
            # HAM warm-up: a few dummy matmuls on garbage SBUF (contents
            # irrelevant, result never read) keep PE busy through the
            # input-DMA wait so the clock gate reaches 8/8 (2.4 GHz) in
            # time for the tail of the real matmul chain. Sized to end
            # just before the s3 DMA semaphore fires (~2.2us window).
            scr = cpool.tile([128, M], bf16)
            nc.vector.memset(scr, 0.0)
            w_ps = wpsum.tile([128, M], f32)
            for _ in range(3):
                nc.tensor.matmul(
                    w_ps, lhsT=scr[:, 0:128], rhs=scr, start=True, stop=True
                )
            # Input DMAs both on the SP HWDGE ring, s3 first (it gates the
            # first matmul; zw is not needed until the third). NOT on the
            # Activation ring: the descriptor-generation slice occupies the
            # issuing engine for ~0.7-1.4us, which on ScalarE would push
            # the ACT table load and the exp chain out by that much. Both
            # are hoisted to the front of the entry block after the
            # TileContext exits.
            s3_sb = cpool.tile([3, NT * 128 + M], bf16)
            i_s3 = nc.sync.dma_start(out=s3_sb, in_=s3[:], single_packet=True)
            zw_sb = cpool.tile([128, NT * Y], bf16)
            i_zw = nc.sync.dma_start(out=zw_sb, in_=zw[:], single_packet=True)

            o_ps = opsum.tile([Y, M], f32)
            for nt in range(NT):
                d_ps = dpsum.tile([128, M], f32, tag=f"d{nt}")
                nc.tensor.matmul(
                    d_ps,
                    lhsT=s3_sb[:, nt * 128 : (nt + 1) * 128],
                    rhs=s3_sb[:, NT * 128 :],
                    start=True,
                    stop=True,
                )
                k_sb = work.tile([128, M], bf16, tag=f"k{nt}")
                nc.scalar.activation(
                    k_sb, d_ps, mybir.ActivationFunctionType.Exp, scale=float(s)
                )
                nc.tensor.matmul(
                    o_ps,
                    lhsT=zw_sb[:, nt * Y : (nt + 1) * Y],
                    rhs=k_sb,
                    start=(nt == 0),
                    stop=(nt == NT - 1),
                )
            o_sb = cpool.tile([Y, M], f32)
            nc.vector.tensor_copy(o_sb, o_ps)
            nc.sync.dma_start(out=o[:], in_=o_sb, single_packet=True)
    _restructure(nc, [i_s3, i_zw])
    _split_multi_waits(nc)
    return nc


def _get_nc(s: float):
    key = ("nc", float(s))
    if key not in _CACHE:
        _CACHE[key] = build_bass(s)
    return _CACHE[key]


def _in_maps_for_group(t, x, zw):
    """Build the 8 per-core input dicts for one sigma-group.

    zw: (B, N, Y) = z[:, :, group] @ W[:, group].T
    """
    import ml_dtypes

    bf16 = ml_dtypes.bfloat16
    in_maps = []
    for core in range(8):
        b, h = core // 2, core % 2
        tb = t[b, h * NHALF : (h + 1) * NHALF, 0]
        xv = x[b, :, 0]
        s3 = np.empty((3, NT * 128 + M), np.float32)
        for nt in range(NT):
            tt = tb[nt * 128 : (nt + 1) * 128]
            s3[0, nt * 128 : (nt + 1) * 128] = tt * tt
            s3[1, nt * 128 : (nt + 1) * 128] = -2.0 * tt
            s3[2, nt * 128 : (nt + 1) * 128] = 1.0
        s3[0, NT * 128 :] = 1.0
        s3[1, NT * 128 :] = xv
        s3[2, NT * 128 :] = xv * xv
        zwm = np.empty((128, NT * Y), np.float32)
        for nt in range(NT):
            lo = h * NHALF + nt * 128
            zwm[:, nt * Y : (nt + 1) * Y] = zw[b, lo : lo + 128, :]
        in_maps.append(
            {
                "s3": s3.astype(bf16),
                "zw": zwm.astype(bf16),
            }
        )
    return in_maps


def _run_group(t, x, zw, s, trace=False):
    from concourse.bass_utils import run_bass_kernel_spmd

    res = run_bass_kernel_spmd(
        _get_nc(s),
        _in_maps_for_group(t, x, zw),
        core_ids=list(range(8)),
        trace=trace,
    )
    out = np.zeros((B, M, Y), np.float32)
    for b in range(B):
        acc = res.results[2 * b]["o"] + res.results[2 * b + 1]["o"]  # (Y, M)
        out[b] = acc.T
    return out, res


def kernel(**inputs):
    t = np.asarray(inputs["t"], np.float32)
    z = np.asarray(inputs["z"], np.float32)
    x = np.asarray(inputs["x"], np.float32)
    sigma = np.asarray(inputs["sigma"], np.float32)
    W = np.asarray(inputs["W"], np.float32)
    bias = np.asarray(inputs["b"], np.float32)

    trace = bool(_CACHE.pop("trace", False))
    out = np.zeros((B, M, Y), np.float32)
    if np.all(sigma == sigma[0]):
        s = -0.5 * float(np.exp(-2.0 * sigma[0]))
        zw = z @ W.T  # (B, N, Y)
        grp_out, res = _run_group(t, x, zw.astype(np.float32), s, trace=trace)
        out += grp_out
        _CACHE["last_results"] = res
    else:
        for val in np.unique(sigma):
            idx = np.nonzero(sigma == val)[0]
            zw = z[:, :, idx] @ W[:, idx].T
            s = -0.5 * float(np.exp(-2.0 * val))
            grp_out, res = _run_group(t, x, zw.astype(np.float32), s, trace=False)
            out += grp_out
    out += bias[None, None, :]
    return out


# revision 18
# speedup vs baseline: 1.0066x; 1.0066x over previous
"""Trainium2 Bass kernel for nn_Decoder (RBF decoder).

Math (shapes: t (4,512,1), z (4,512,128), x (4,512,1), sigma (128,),
W (2,128), b (2,)):
    diff[b,n,m] = x[b,m] - t[b,n]                  (XD=1, sum(-1) trivial)
    K[b,n,m,c]  = exp(-0.5 * (diff/exp(sigma[c]))^2)
    y[b,m,c]    = sum_n z[b,n,c] * K[b,n,m,c]
    out[b,m,:]  = y[b,m,:] @ W.T + b

When all sigma[c] are equal (they are zeros for this problem), K is
channel-independent, so W can be folded into z up front:
    zw[b] = z[b] @ W.T            (host, (N,2) per batch — tiny)
    out[b].T = sum_n zw[b,n,:]^T K[b][n,:],  K[b] = exp(s * (x_m - t_n)^2),
    s = -0.5*exp(-2*sigma).

Device mapping (8 cores, SPMD): core k handles batch b=k//2, n-half
h=k%2 (n-slice of 256 = 2 tiles of 128 partitions). Per core:
  - d2[n,m] = (x_m - t_n)^2 is produced directly in PSUM by a rank-3
    bf16 matmul: lhsT = [t^2; -2t; 1] (3,128 per n-tile), rhs =
    [1; x; x^2] (3,512), so no x-broadcast DMA and no Square pass.
    Host pre-rounds everything to bf16; products are exact in the fp32
    PSUM accumulator, so the only error is input rounding (~3e-3 rel
    on the final output, vs the 2e-2 gate).
  - ScalarE: K = exp(s * d2) read straight from PSUM, written to SBUF
    as bf16 (s baked as the ACT scale immediate). The ACT table load
    (~1.3us) is hoisted by the engine queue to run during the input
    DMA window.
  - PE: psum(2,512) += matmul(lhsT=zw bf16 (128,2), rhs=K bf16
    (128,512)) accumulated over the 2 n-tiles. bf16 single-pass
    matmuls (fp32 would be LOW_HIGH dual-issue, ~2x the cost). No
    HAM warm-up: with only 4 matmuls the cold-clock penalty is far
    smaller than the ~4.5us a warm-up string costs.
  - DVE evicts the psum -> SBUF, one DMA out (2,512) = out[b].T
    partial.
Host sums the two n-half partials per batch, transposes, adds bias b.

Input DMAs ride the two independent HWDGE rings (nc.sync / nc.scalar)
so they overlap; nothing touches the GpSimd SWDGE path (its drain tail
is ~5us).

Sync-wait discipline: this container's walrus allows a single on_wait
per instruction ("Too many sync wait commands"), so _split_multi_waits
rewrites the scheduled BIR, hoisting extra waits onto same-engine NOPs
placed immediately before the instruction (same-engine program order
preserves semantics).

General (non-uniform) sigma falls back to grouping channels by unique
sigma value (zw_g from just that group's channels, s_g baked into a
per-group NEFF) and summing the group outputs, which is exact since the
output is linear in z. The graded instance has sigma == 0: one group.
"""

import numpy as np

B, N, M, C, Y = 4, 512, 512, 128, 2
NHALF = N // 2  # n-slice per core
NT = NHALF // 128  # n-tiles of 128 per core

_CACHE = {}


def _split_multi_waits(nc):
    import concourse.mybir as mybir

    for fn in nc.m.functions:
        for blk in fn.blocks:
            il = blk.instructions
            new = []
            for inst in il:
                si = inst.sync_info
                if si is not None and si.on_wait is not None and len(si.on_wait) > 1:
                    waits = list(si.on_wait)
                    for j, w in enumerate(waits[:-1]):
                        new.append(
                            mybir.InstNoOp(
                                name=f"{inst.name}-w{j}",
                                engine=inst.engine,
                                sync_info=mybir.SyncInfo(on_wait=[w], on_update=[]),
                                bass_nofuse=True,
                            )
                        )
                    si.on_wait = [waits[-1]]
                    inst.sync_info = si
                new.append(inst)
            il[:] = new


def _restructure(nc, dma_insts):
    """Post-build BIR surgery to pull fixed latency off the critical path.

    1. Hoist the input DMAs to the very FRONT of their engine's stream
       in the 'main' entry block. The walrus NEFF prologue (start
       barrier + register loads, ~5.6us) runs per engine ahead of
       'main'; dispatching the DMAs first overlaps their ~2.2us
       fixed descriptor/doorbell/HBM latency with the rest of the
       entry sequence instead of paying it serially in the body.
    2. Drop the TileContext entry barrier (per-engine InstDrain +
       EventSemaphore handshake on S151/S152). It only ordered the
       Pool const-tile memsets against the body; the first consumer
       (the exp bias read) runs >2us after the memsets regardless,
       and the input DMAs must not sit behind a Drain (an InstDrain
       waits for the engine's outstanding DMA descriptors to retire).
    3. Drop the end-block handshake that FOLLOWS the semaphore-reset
       InstISA (round B). The walrus end-of-iteration all-engine
       barrier already orders the resets before the next execution.

    Iteration safety: the walrus inter-iteration barrier keeps the
    hoisted DMA writes of run N+1 after all reads of run N, and the
    kept pre-reset handshake (round A) still quiesces every engine —
    including the output DMA receipt — before the InstISA resets.
    """
    import concourse.mybir as mybir

    fn = nc.m.functions[0]
    main, end = fn.blocks[0], fn.blocks[-1]
    dma = [i.ins if hasattr(i, "ins") else i for i in dma_insts]
    names = {i.name for i in dma}
    for blk in fn.blocks:
        blk.instructions[:] = [i for i in blk.instructions if i.name not in names]
    main.instructions[:] = [
        i
        for i in main.instructions
        if not isinstance(i, (mybir.InstDrain, mybir.InstEventSemaphore))
    ]
    il = main.instructions
    for inst in reversed(dma):
        si = inst.sync_info
        assert si is None or not si.on_wait, f"hoisted DMA has waits: {inst.name}"
        idx = next(j for j, m in enumerate(il) if m.engine == inst.engine)
        il.insert(idx, inst)
    # End block: keep only the SP stream up to and including its first
    # InstDrain (the multi-wait drain gating NEFF completion on the
    # output-DMA receipt). The pre-reset handshake, the InstISA
    # semaphore resets, and the post-reset handshake all go: the
    # runtime resets every semaphore between executions anyway, and
    # the walrus end-of-iteration all-engine barrier orders runs.
    il = end.instructions
    kept = []
    sp_done = False
    for m in il:
        if str(m.engine).endswith("SP") and not sp_done:
            kept.append(m)
            if isinstance(m, mybir.InstDrain):
                sp_done = True
    il[:] = kept


def build_bass(s: float):
    """Build the per-core Bass module; `s` (= -0.5*exp(-2*sigma)) is baked
    into the exp activation as a float immediate."""
    import concourse.bass as bass
    import concourse.mybir as mybir
    import concourse.tile as tile

    f32 = mybir.dt.float32
    bf16 = mybir.dt.bfloat16
    nc = bass.Bass(enable_partition_id=False)
    # s3 = [lhsT3 tile0 | lhsT3 tile1 | rhs3]: rows [t^2;-2t;1 | 1;x;x^2]
    s3 = nc.dram_tensor("s3", (3, NT * 128 + M), bf16, kind="ExternalInput")
    # zw cols per n-tile: (128, NT*Y)
    zw = nc.dram_tensor("zw", (128, NT * Y), bf16, kind="ExternalInput")
    o = nc.dram_tensor("o", (Y, M), f32, kind="ExternalOutput")

    with tile.TileContext(nc) as tc:
        with (
            tc.tile_pool(name="const", bufs=1) as cpool,
            tc.tile_pool(name="work", bufs=2) as work,
            tc.tile_pool(name="dpsum", bufs=2, space="PSUM") as dpsum,
            tc.tile_pool(name="opsum", bufs=1, space="PSUM") as opsum,
        ):
            # No HAM warm-up: PE cannot start dummy work before ~6us
            # (post-prologue), so the 8/8 clock would arrive at ~9.4us at
            # the earliest — after nearly the whole real matmul chain.
            # Measured: warm-up dummies only delayed the chain (14539 vs
            # 13950 ns).
            # Input DMAs both on the SP HWDGE ring, s3 first (it gates the
            # first matmul; zw is not needed until the third). NOT on the
            # Activation ring: the descriptor-generation slice occupies the
            # issuing engine for ~0.7-1.4us, which on ScalarE would push
            # the ACT table load and the exp chain out by that much. Both
            # are hoisted to the front of the entry block after the
            # TileContext exits.
            s3_sb = cpool.tile([3, NT * 128 + M], bf16)
            i_s3 = nc.scalar.dma_start(out=s3_sb, in_=s3[:], single_packet=True)
            zw_sb = cpool.tile([128, NT * Y], bf16)
            i_zw = nc.sync.dma_start(out=zw_sb, in_=zw[:], single_packet=True)

            o_ps = opsum.tile([Y, M], f32)
            for nt in range(NT):
                d_ps = dpsum.tile([128, M], f32, tag=f"d{nt}")
                nc.tensor.matmul(
                    d_ps,
                    lhsT=s3_sb[:, nt * 128 : (nt + 1) * 128],
                    rhs=s3_sb[:, NT * 128 :],
                    start=True,
                    stop=True,
                )
                k_sb = work.tile([128, M], bf16, tag=f"k{nt}")
                nc.scalar.activation(
                    k_sb, d_ps, mybir.ActivationFunctionType.Exp, scale=float(s)
                )
                nc.tensor.matmul(
                    o_ps,
                    lhsT=zw_sb[:, nt * Y : (nt + 1) * Y],
                    rhs=k_sb,
                    start=(nt == 0),
                    stop=(nt == NT - 1),
                )
            o_sb = cpool.tile([Y, M], f32)
            nc.vector.tensor_copy(o_sb, o_ps)
            nc.sync.dma_start(out=o[:], in_=o_sb, single_packet=True)
    _restructure(nc, [i_s3, i_zw])
    _split_multi_waits(nc)
    return nc


def _get_nc(s: float):
    key = ("nc", float(s))
    if key not in _CACHE:
        _CACHE[key] = build_bass(s)
    return _CACHE[key]


def _in_maps_for_group(t, x, zw):
    """Build the 8 per-core input dicts for one sigma-group.

    zw: (B, N, Y) = z[:, :, group] @ W[:, group].T
    """
    import ml_dtypes

    bf16 = ml_dtypes.bfloat16
    in_maps = []
    for core in range(8):
        b, h = core // 2, core % 2
        tb = t[b, h * NHALF : (h + 1) * NHALF, 0]
        xv = x[b, :, 0]
        s3 = np.empty((3, NT * 128 + M), np.float32)
        for nt in range(NT):
            tt = tb[nt * 128 : (nt + 1) * 128]
            s3[0, nt * 128 : (nt + 1) * 128] = tt * tt
            s3[1, nt * 128 : (nt + 1) * 128] = -2.0 * tt
            s3[2, nt * 128 : (nt + 1) * 128] = 1.0
        s3[0, NT * 128 :] = 1.0
        s3[1, NT * 128 :] = xv
        s3[2, NT * 128 :] = xv * xv
        zwm = np.empty((128, NT * Y), np.float32)
        for nt in range(NT):
            lo = h * NHALF + nt * 128
            zwm[:, nt * Y : (nt + 1) * Y] = zw[b, lo : lo + 128, :]
        in_maps.append(
            {
                "s3": s3.astype(bf16),
                "zw": zwm.astype(bf16),
            }
        )
    return in_maps


def _run_group(t, x, zw, s, trace=False):
    from concourse.bass_utils import run_bass_kernel_spmd

    res = run_bass_kernel_spmd(
        _get_nc(s),
        _in_maps_for_group(t, x, zw),
        core_ids=list(range(8)),
        trace=trace,
    )
    out = np.zeros((B, M, Y), np.float32)
    for b in range(B):
        acc = res.results[2 * b]["o"] + res.results[2 * b + 1]["o"]  # (Y, M)
        out[b] = acc.T
    return out, res


def kernel(**inputs):
    t = np.asarray(inputs["t"], np.float32)
    z = np.asarray(inputs["z"], np.float32)
    x = np.asarray(inputs["x"], np.float32)
    sigma = np.asarray(inputs["sigma"], np.float32)
    W = np.asarray(inputs["W"], np.float32)
    bias = np.asarray(inputs["b"], np.float32)

    trace = bool(_CACHE.pop("trace", False))
    out = np.zeros((B, M, Y), np.float32)
    if np.all(sigma == sigma[0]):
        s = -0.5 * float(np.exp(-2.0 * sigma[0]))
        zw = z @ W.T  # (B, N, Y)
        grp_out, res = _run_group(t, x, zw.astype(np.float32), s, trace=trace)
        out += grp_out
        _CACHE["last_results"] = res
    else:
        for val in np.unique(sigma):
            idx = np.nonzero(sigma == val)[0]
            zw = z[:, :, idx] @ W[:, idx].T
            s = -0.5 * float(np.exp(-2.0 * val))
            grp_out, res = _run_group(t, x, zw.astype(np.float32), s, trace=False)
            out += grp_out
    out += bias[None, None, :]
    return out
